# revision 16
# baseline (speedup 1.0000x reference)
"""ChannelMerger kernel for 8x Trainium2 NeuronCores (SPMD, data-parallel over batch).

Reference computation (all f32):
    emb = fourier_emb(positions)            # [C, D]   D = 2048
    sub_heads = heads[subject]              # [B, O, D]
    scores = einsum('cd,bod->boc', emb, sub_heads)
    weights = softmax(scores, axis=2)       # over C
    out = einsum('bct,boc->bot', meg, weights)

Device strategy per core (16 batches each):
  - Fourier embedding computed on-chip in transposed layout embT [D, C]
    (DVE for the phase args with mod-2pi range reduction via the +2^23
    round-to-nearest trick, ACT Sin LUT for sin/cos).
  - Per-subject scoresT [C, O] = embT.T @ headsT via PE (fp32r), softmax
    across the C partition axis: ACT Exp, ones-vector matmul for the sums,
    DVE reciprocal, GpSimd partition-broadcast, DVE multiply.
  - Normalized weightsT staged to a DRAM scratch table; each batch slot
    gathers its subject's weightsT via indirect DMA (subject indices are
    turned into row indices on-chip).
  - Main einsum: out[b] = weightsT.T @ meg[b] on PE (fp32r), PSUM -> SBUF
    copies on DVE/ACT, DMA out.

fp32r is the TRN2 TensorE fp32 mode with 11-bit mantissa (TF32-like) running
at full 1 cycle/row, giving ~1.5e-4 relative error vs the f32 reference.
"""

import math

import numpy as np

import concourse.bacc as bacc
import concourse.bass as bass
import concourse.mybir as mybir
import concourse.tile as tile
from concourse.tile_rust import add_dep_helper
from concourse.bass_utils import run_bass_kernel_spmd

P = 128
B, C, T = 128, 273, 3072
S, O, D = 4, 270, 2048
NCORES = 8
BL = B // NCORES  # 16 batches per core

KC = 3   # C k-tiles: 128, 128, 17
KD = D // P  # 16 d-tiles
MO = 3   # O chunks: 128, 128, 14
NT = T // 512  # 6

MARGIN = 0.2
WIDTH = 1.0 + 2.0 * MARGIN
TWO_PI = 2.0 * math.pi
MAGIC = float(2 ** 23)  # round-to-nearest-even trick (valid for u >= 0)

F32 = mybir.dt.float32
F32R = mybir.dt.float32r
F16 = mybir.dt.float16
I32 = mybir.dt.int32
DT_MM = F16   # main einsum operand dtype (weightsT, meg)
DT_SC = F16   # scores matmul operand dtype (embT, headsT)

SCRATCH_ROWS = S * KC * P  # 1536; row (s, kc, p) = weightsT[c = kc*128+p, :] of subject s


def _dedupe_ldweights(nc):
    """Drop redundant InstLdweights: consecutive loads of the identical weights
    slice with only matmuls/evsems in between keep only the first load. The
    matmuls are already non-self-loading (ldweights=False), so later matmuls
    simply reuse the array state. Only waits/updates-free LDWs are dropped."""
    neutral = (mybir.InstMatmult, mybir.InstEventSemaphore)
    dropped = 0
    for bb in nc.m.functions[0].blocks:
        insts = bb.instructions
        out = []
        last_sig = None
        for i in insts:
            if isinstance(i, mybir.InstLdweights):
                si = i.sync_info
                clean = not (si and (si.on_wait or si.on_update))
                sig = (str(i.ins[0]), str(getattr(i, "perf_mode", None)),
                       str(getattr(i, "is_transpose", None)))
                if clean and sig == last_sig and "wsel" in sig[0]:
                    dropped += 1
                    continue
                last_sig = sig
            elif not isinstance(i, neutral):
                if getattr(i, "engine", None) == mybir.EngineType.PE:
                    last_sig = None
            out.append(i)
        if dropped:
            bb.instructions = out
    return dropped


def _pk(kc):
    return min(P, C - kc * P)


def _pmo(mo):
    return min(P, O - mo * P)


def build():
    nc = bacc.Bacc()
    meg_l = nc.declare_dram_parameter("meg_l", [BL, C, T], F16, isOutputAFalse=False) if False else nc.declare_dram_parameter("meg_l", [BL, C, T], F16, isOutput=False)
    headsT = nc.declare_dram_parameter("headsT", [S, D, O], F16, isOutput=False)
    posflat = nc.declare_dram_parameter("posflat", [1, 2 * C], F32, isOutput=False)
    ftab = nc.declare_dram_parameter("ftab", [2, KD * P // 2], F32, isOutput=False)  # [2,1024]
    iota = nc.declare_dram_parameter("iota", [P, 1], F32, isOutput=False)
    subj = nc.declare_dram_parameter("subj", [1, BL], I32, isOutput=False)
    out_l = nc.declare_dram_parameter("out_l", [BL, O, T], F32, isOutput=True)

    with tile.TileContext(nc) as tc:
        with (
            tc.tile_pool(name="const", bufs=1) as cp,
            tc.tile_pool(name="dram", bufs=1, space="DRAM") as dp,
            tc.tile_pool(name="rhs", bufs=5) as rhs_p,
            tc.tile_pool(name="wsel", bufs=3) as wsel_p,
            tc.tile_pool(name="ot", bufs=10) as ot_p,
        ):
            scratch = dp.tile([SCRATCH_ROWS, O], DT_MM)

            # ---------------- phase 0: constants ----------------
            posB = cp.tile([P, 2 * C], F32)
            nc.sync.dma_start(out=posB[:], in_=posflat[0:1, :].partition_broadcast(P))
            posP = cp.tile([P, 2 * C], F32)
            nc.vector.tensor_scalar_add(out=posP[:], in0=posB[:], scalar1=MARGIN)
            posPr = posP[:].rearrange("p (c two) -> p two c", two=2)
            px = posPr[:, 0]  # [P, C] (stride-2 free)
            py = posPr[:, 1]

            fi_all = cp.tile([P, KD // 2], F32)  # [128, 8]
            fj_all = cp.tile([P, KD // 2], F32)
            nc.sync.dma_start(
                out=fi_all[:], in_=ftab[0:1, :].rearrange("one (k p) -> (one p) k", p=P)
            )
            nc.sync.dma_start(
                out=fj_all[:], in_=ftab[1:2, :].rearrange("one (k p) -> (one p) k", p=P)
            )

            iota_t = cp.tile([P, 1], F32)
            nc.sync.dma_start(out=iota_t[:], in_=iota[:])

            subj_t = cp.tile([P, BL], I32)
            nc.sync.dma_start(out=subj_t[:], in_=subj[0:1, :].partition_broadcast(P))
            subjB = cp.tile([P, BL], F32)
            nc.vector.tensor_copy(out=subjB[:], in_=subj_t[:])

            # gather row indices: idx[p, kc*BL + j] = 384*subject[j] + 128*kc + p
            idx_f = cp.tile([P, KC * BL], F32)
            for kc in range(KC):
                nc.vector.tensor_scalar(
                    out=idx_f[:, kc * BL : (kc + 1) * BL],
                    in0=subjB[:],
                    scalar1=float(KC * P),
                    scalar2=float(P * kc),
                    op0=mybir.AluOpType.mult,
                    op1=mybir.AluOpType.add,
                )
            nc.vector.tensor_tensor(
                out=idx_f[:],
                in0=idx_f[:],
                in1=iota_t[:, 0:1].to_broadcast([P, KC * BL]),
                op=mybir.AluOpType.add,
            )
            idx_i = cp.tile([P, KC * BL], I32)
            nc.vector.tensor_copy(out=idx_i[:], in_=idx_f[:])

            ones_col = cp.tile([P, 1], F32)
            nc.vector.memset(ones_col[:], 1.0)

            # ---------------- phase 1: embT [D, C] (fp32r) ----------------
            embT = cp.tile([P, KD, C], DT_SC)
            with tc.tile_pool(name="etmp", bufs=2) as tp:
                for kd in range(KD // 2):
                    t1 = tp.tile([P, C], F32, tag="t1")
                    nc.vector.tensor_scalar(
                        out=t1[:], in0=px, scalar1=fi_all[:, kd : kd + 1],
                        scalar2=None, op0=mybir.AluOpType.mult,
                    )
                    u = tp.tile([P, C], F32, tag="u")
                    nc.vector.tensor_scalar(
                        out=u[:], in0=py, scalar1=fj_all[:, kd : kd + 1],
                        scalar2=None, op0=mybir.AluOpType.mult,
                    )
                    nc.vector.tensor_tensor(
                        out=u[:], in0=u[:], in1=t1[:], op=mybir.AluOpType.add
                    )
                    # sin half: embT[kd + 8] = sin(2*pi*(u - rne(u)))
                    rn = tp.tile([P, C], F32, tag="rn")
                    nc.vector.tensor_scalar(
                        out=rn[:], in0=u[:], scalar1=MAGIC, scalar2=MAGIC,
                        op0=mybir.AluOpType.add, op1=mybir.AluOpType.subtract,
                    )
                    fr = tp.tile([P, C], F32, tag="fr")
                    nc.vector.tensor_tensor(
                        out=fr[:], in0=u[:], in1=rn[:], op=mybir.AluOpType.subtract
                    )
                    nc.scalar.activation(
                        embT[:, kd + KD // 2, :], fr[:],
                        mybir.ActivationFunctionType.Sin, bias=0.0, scale=TWO_PI,
                    )
                    # cos half: embT[kd] = sin(2*pi*(uc - rne(uc))), uc = u + 0.25
                    vc = tp.tile([P, C], F32, tag="vc")
                    nc.vector.tensor_scalar_add(out=vc[:], in0=u[:], scalar1=0.25)
                    rc = tp.tile([P, C], F32, tag="rc")
                    nc.vector.tensor_scalar(
                        out=rc[:], in0=vc[:], scalar1=MAGIC, scalar2=MAGIC,
                        op0=mybir.AluOpType.add, op1=mybir.AluOpType.subtract,
                    )
                    fc = tp.tile([P, C], F32, tag="fc")
                    nc.vector.tensor_tensor(
                        out=fc[:], in0=vc[:], in1=rc[:], op=mybir.AluOpType.subtract
                    )
                    nc.scalar.activation(
                        embT[:, kd, :], fc[:],
                        mybir.ActivationFunctionType.Sin, bias=0.0, scale=TWO_PI,
                    )

            # ---------------- phase 2: per-subject softmax weightsT ----------------
            with (
                tc.tile_pool(name="hT", bufs=S * KD) as h_p,
                tc.tile_pool(name="E", bufs=S) as e_p,
                tc.tile_pool(name="wT", bufs=2) as w_p,
                tc.tile_pool(name="smisc", bufs=4) as sm_p,
                tc.tile_pool(name="pssc", bufs=6, space="PSUM") as pssc_p,
            ):
                hts = {}
                for s in range(S):
                    for kd in range(KD):
                        hT = h_p.tile([P, O], F16, tag="hT", name=f"hT_{s}_{kd}")
                        nc.sync.dma_start(out=hT[:], in_=headsT[s, kd * P : (kd + 1) * P, :])
                        hts[(s, kd)] = hT
                e_ts = [e_p.tile([P, KC, O], F32, tag="E", name=f"E_{s}") for s in range(S)]
                prev_smm = None
                for mc in range(KC):
                    pk = _pk(mc)
                    pscs = [
                        pssc_p.tile([P, O], F32, space="PSUM", tag="psc", name=f"psc_{mc}_{s}")
                        for s in range(S)
                    ]
                    kd_order = [x for p in range(KD // 2) for x in (p, p + KD // 2)]
                    for ki, kd in enumerate(kd_order):
                        for s in range(S):
                            mm = nc.tensor.matmul(
                                out=pscs[s][:pk, :],
                                lhsT=embT[:, kd, mc * P : mc * P + pk],
                                rhs=hts[(s, kd)][:],
                                start=(ki == 0),
                                stop=(ki == KD - 1),
                            )
                            if prev_smm is not None:
                                add_dep_helper(mm.ins, prev_smm.ins, sync=False, reason="pe-order")
                            prev_smm = mm
                    for s in range(S):
                        nc.scalar.activation(
                            e_ts[s][:pk, mc, :], pscs[s][:pk, :],
                            mybir.ActivationFunctionType.Exp,
                        )
                with tc.tile_pool(name="pssum", bufs=1, space="PSUM") as pssum_p:
                    for s in range(S):
                        e_t = e_ts[s]
                        sums = pssum_p.tile([1, O], F32, space="PSUM", tag="sums", name=f"sums_{s}")
                        for mc in range(KC):
                            pk = _pk(mc)
                            nc.tensor.matmul(
                                out=sums[:, :],
                                lhsT=ones_col[:pk, :],
                                rhs=e_t[:pk, mc, :],
                                start=(mc == 0),
                                stop=(mc == KC - 1),
                            )
                        srow = sm_p.tile([1, O], F32, tag="srow", name=f"srow_{s}")
                        nc.vector.tensor_copy(out=srow[:], in_=sums[:])
                        rrow = sm_p.tile([1, O], F32, tag="rrow", name=f"rrow_{s}")
                        nc.vector.reciprocal(out=rrow[:], in_=srow[:])
                        recipB = sm_p.tile([P, O], F32, tag="recipB", name=f"recipB_{s}")
                        nc.gpsimd.partition_broadcast(recipB[:], rrow[:])
                        w_t = w_p.tile([P, KC, O], DT_MM, tag="wT", name=f"wT_{s}")
                        for mc in range(KC):
                            pk = _pk(mc)
                            nc.vector.tensor_tensor(
                                out=w_t[:pk, mc, :],
                                in0=e_t[:pk, mc, :],
                                in1=recipB[:pk, :],
                                op=mybir.AluOpType.mult,
                            )
                            nc.sync.dma_start(
                                out=scratch[s * KC * P + mc * P : s * KC * P + mc * P + pk, :],
                                in_=w_t[:pk, mc, :],
                            )

            # ---------------- phase 3: main einsum per batch slot ----------------
            psm_ctx = tc.tile_pool(name="psmain", bufs=8, space="PSUM")
            psm_p = psm_ctx.__enter__()
            prev_mm = None
            for j in range(BL):
                wsel = wsel_p.tile([P, KC, O], DT_MM, tag="wsel")
                for kc in range(KC):
                    pk = _pk(kc)
                    col = kc * BL + j
                    nc.gpsimd.indirect_dma_start(
                        out=wsel[:pk, kc, :],
                        out_offset=None,
                        in_=scratch[:],
                        in_offset=bass.IndirectOffsetOnAxis(
                            ap=idx_i[:pk, col : col + 1], axis=0
                        ),
                    )
                rhs = rhs_p.tile([P, KC, T], DT_MM, tag="rhs")
                for kc in range(KC):
                    pk = _pk(kc)
                    nc.gpsimd.dma_start(
                        out=rhs[:pk, kc, :], in_=meg_l[j, kc * P : kc * P + pk, :]
                    )
                for mo in range(MO):
                    pmo = _pmo(mo)
                    pos = [
                        psm_p.tile([P, 512], F32, space="PSUM", tag="po", name=f"po_{j}_{mo}_{n}")
                        for n in range(NT)
                    ]
                    # kc outer / nt inner: one weight set serves NT consecutive matmuls
                    for kc in range(KC):
                        pk = _pk(kc)
                        for nt in range(NT):
                            mm = nc.tensor.matmul(
                                out=pos[nt][:pmo, :],
                                lhsT=wsel[:pk, kc, mo * P : mo * P + pmo],
                                rhs=rhs[:pk, kc, nt * 512 : (nt + 1) * 512],
                                start=(kc == 0),
                                stop=(kc == KC - 1),
                            )
                            # pin PE order kc-outer/nt-inner so the weight
                            # dedup pass sees adjacent identical LDWEIGHTS
                            if prev_mm is not None:
                                add_dep_helper(mm.ins, prev_mm.ins, sync=False, reason="pe-order")
                            prev_mm = mm
                    for nt in range(NT):
                        ot = ot_p.tile([P, 512], F32, tag="ot")
                        # alternate copy engines (DVE is mostly idle) and
                        # spread the out-DMAs over both HWDGE rings
                        if nt % 2 == 0:
                            nc.vector.tensor_copy(out=ot[:pmo, :], in_=pos[nt][:pmo, :])
                            nc.sync.dma_start(
                                out=out_l[j, mo * P : mo * P + pmo, nt * 512 : (nt + 1) * 512],
                                in_=ot[:pmo, :],
                            )
                        else:
                            nc.scalar.copy(out=ot[:pmo, :], in_=pos[nt][:pmo, :])
                            nc.scalar.dma_start(
                                out=out_l[j, mo * P : mo * P + pmo, nt * 512 : (nt + 1) * 512],
                                in_=ot[:pmo, :],
                            )

            psm_ctx.__exit__(None, None, None)

    import os as _os
    if not _os.environ.get("NO_DEDUP"):
        _dedupe_ldweights(nc)
    nc.compile()
    return nc


_NC = None


def _host_inputs():
    """Input arrays that do not depend on the problem inputs."""
    d = np.arange(KD * P // 2)
    ftab = np.stack(
        [
            ((d // 32).astype(np.float64) / WIDTH).astype(np.float32),
            ((d % 32).astype(np.float64) / WIDTH).astype(np.float32),
        ]
    )
    iota = np.arange(P, dtype=np.float32).reshape(P, 1)
    return ftab, iota


def kernel(meg, positions, heads, subject, _trace=False, **_unused):
    global _NC
    if _NC is None:
        _NC = build()
    nc = _NC

    meg = np.ascontiguousarray(meg, dtype=np.float16)
    positions = np.ascontiguousarray(positions, dtype=np.float32)
    headsT = np.ascontiguousarray(np.transpose(heads, (0, 2, 1)), dtype=np.float16)
    subject = np.asarray(subject).astype(np.int32)
    ftab, iota = _host_inputs()
    posflat = positions.reshape(1, 2 * C)

    in_maps = []
    for i in range(NCORES):
        in_maps.append(
            {
                "meg_l": meg[i * BL : (i + 1) * BL],
                "headsT": headsT,
                "posflat": posflat,
                "ftab": ftab,
                "iota": iota,
                "subj": subject[i * BL : (i + 1) * BL].reshape(1, BL),
            }
        )

    res = run_bass_kernel_spmd(nc, in_maps, list(range(NCORES)), trace=_trace)
    out = np.concatenate([res.results[i]["out_l"] for i in range(NCORES)], axis=0)
    if _trace:
        return out, res
    return out


# revision 18
# speedup vs baseline: 1.0056x; 1.0056x over previous
"""ChannelMerger kernel for 8x Trainium2 NeuronCores (SPMD, data-parallel over batch).

Reference computation (all f32):
    emb = fourier_emb(positions)            # [C, D]   D = 2048
    sub_heads = heads[subject]              # [B, O, D]
    scores = einsum('cd,bod->boc', emb, sub_heads)
    weights = softmax(scores, axis=2)       # over C
    out = einsum('bct,boc->bot', meg, weights)

Device strategy per core (16 batches each):
  - Fourier embedding computed on-chip in transposed layout embT [D, C]
    (DVE for the phase args with mod-2pi range reduction via the +2^23
    round-to-nearest trick, ACT Sin LUT for sin/cos).
  - Per-subject scoresT [C, O] = embT.T @ headsT via PE (fp32r), softmax
    across the C partition axis: ACT Exp, ones-vector matmul for the sums,
    DVE reciprocal, GpSimd partition-broadcast, DVE multiply.
  - Normalized weightsT staged to a DRAM scratch table; each batch slot
    gathers its subject's weightsT via indirect DMA (subject indices are
    turned into row indices on-chip).
  - Main einsum: out[b] = weightsT.T @ meg[b] on PE (fp32r), PSUM -> SBUF
    copies on DVE/ACT, DMA out.

fp32r is the TRN2 TensorE fp32 mode with 11-bit mantissa (TF32-like) running
at full 1 cycle/row, giving ~1.5e-4 relative error vs the f32 reference.
"""

import math

import numpy as np

import concourse.bacc as bacc
import concourse.bass as bass
import concourse.mybir as mybir
import concourse.tile as tile
from concourse.tile_rust import add_dep_helper
from concourse.bass_utils import run_bass_kernel_spmd

P = 128
B, C, T = 128, 273, 3072
S, O, D = 4, 270, 2048
NCORES = 8
BL = B // NCORES  # 16 batches per core

KC = 3   # C k-tiles: 128, 128, 17
KD = D // P  # 16 d-tiles
MO = 3   # O chunks: 128, 128, 14
NT = T // 512  # 6

MARGIN = 0.2
WIDTH = 1.0 + 2.0 * MARGIN
TWO_PI = 2.0 * math.pi
MAGIC = float(2 ** 23)  # round-to-nearest-even trick (valid for u >= 0)

F32 = mybir.dt.float32
F32R = mybir.dt.float32r
F16 = mybir.dt.float16
I32 = mybir.dt.int32
DT_MM = F16   # main einsum operand dtype (weightsT, meg)
DT_SC = F16   # scores matmul operand dtype (embT, headsT)

SCRATCH_ROWS = S * KC * P  # 1536; row (s, kc, p) = weightsT[c = kc*128+p, :] of subject s


def _dedupe_ldweights(nc):
    """Drop redundant InstLdweights: consecutive loads of the identical weights
    slice with only matmuls/evsems in between keep only the first load. The
    matmuls are already non-self-loading (ldweights=False), so later matmuls
    simply reuse the array state. Only waits/updates-free LDWs are dropped."""
    neutral = (mybir.InstMatmult, mybir.InstEventSemaphore)
    dropped = 0
    for bb in nc.m.functions[0].blocks:
        insts = bb.instructions
        out = []
        last_sig = None
        for i in insts:
            if isinstance(i, mybir.InstLdweights):
                si = i.sync_info
                clean = not (si and (si.on_wait or si.on_update))
                sig = (str(i.ins[0]), str(getattr(i, "perf_mode", None)),
                       str(getattr(i, "is_transpose", None)))
                if clean and sig == last_sig and "wsel" in sig[0]:
                    dropped += 1
                    continue
                last_sig = sig
            elif not isinstance(i, neutral):
                if getattr(i, "engine", None) == mybir.EngineType.PE:
                    last_sig = None
            out.append(i)
        if dropped:
            bb.instructions = out
    return dropped


def _pk(kc):
    return min(P, C - kc * P)


def _pmo(mo):
    return min(P, O - mo * P)


def build():
    nc = bacc.Bacc()
    meg_l = nc.declare_dram_parameter("meg_l", [BL, C, T], F16, isOutput=False)
    headsT = nc.declare_dram_parameter("headsT", [S, D, O], F16, isOutput=False)
    posflat = nc.declare_dram_parameter("posflat", [1, 2 * C], F32, isOutput=False)
    ftab = nc.declare_dram_parameter("ftab", [2, KD * P // 2], F32, isOutput=False)  # [2,1024]
    iota = nc.declare_dram_parameter("iota", [P, 1], F32, isOutput=False)
    subj = nc.declare_dram_parameter("subj", [1, BL], I32, isOutput=False)
    out_l = nc.declare_dram_parameter("out_l", [BL, O, T], F32, isOutput=True)

    with tile.TileContext(nc) as tc:
        with (
            tc.tile_pool(name="const", bufs=1) as cp,
            tc.tile_pool(name="dram", bufs=1, space="DRAM") as dp,
            tc.tile_pool(name="rhs", bufs=5) as rhs_p,
            tc.tile_pool(name="wsel", bufs=3) as wsel_p,
            tc.tile_pool(name="ot", bufs=10) as ot_p,
        ):
            scratch = dp.tile([SCRATCH_ROWS, O], DT_MM)

            # ---------------- phase 0: constants ----------------
            posB = cp.tile([P, 2 * C], F32)
            nc.sync.dma_start(out=posB[:], in_=posflat[0:1, :].partition_broadcast(P))
            posP = cp.tile([P, 2 * C], F32)
            nc.vector.tensor_scalar_add(out=posP[:], in0=posB[:], scalar1=MARGIN)
            posPr = posP[:].rearrange("p (c two) -> p two c", two=2)
            px = posPr[:, 0]  # [P, C] (stride-2 free)
            py = posPr[:, 1]

            fi_all = cp.tile([P, KD // 2], F32)  # [128, 8]
            fj_all = cp.tile([P, KD // 2], F32)
            nc.sync.dma_start(
                out=fi_all[:], in_=ftab[0:1, :].rearrange("one (k p) -> (one p) k", p=P)
            )
            nc.sync.dma_start(
                out=fj_all[:], in_=ftab[1:2, :].rearrange("one (k p) -> (one p) k", p=P)
            )

            iota_t = cp.tile([P, 1], F32)
            nc.sync.dma_start(out=iota_t[:], in_=iota[:])

            subj_t = cp.tile([P, BL], I32)
            nc.sync.dma_start(out=subj_t[:], in_=subj[0:1, :].partition_broadcast(P))
            subjB = cp.tile([P, BL], F32)
            nc.vector.tensor_copy(out=subjB[:], in_=subj_t[:])

            # gather row indices: idx[p, kc*BL + j] = 384*subject[j] + 128*kc + p
            idx_f = cp.tile([P, KC * BL], F32)
            for kc in range(KC):
                nc.vector.tensor_scalar(
                    out=idx_f[:, kc * BL : (kc + 1) * BL],
                    in0=subjB[:],
                    scalar1=float(KC * P),
                    scalar2=float(P * kc),
                    op0=mybir.AluOpType.mult,
                    op1=mybir.AluOpType.add,
                )
            nc.vector.tensor_tensor(
                out=idx_f[:],
                in0=idx_f[:],
                in1=iota_t[:, 0:1].to_broadcast([P, KC * BL]),
                op=mybir.AluOpType.add,
            )
            idx_i = cp.tile([P, KC * BL], I32)
            nc.vector.tensor_copy(out=idx_i[:], in_=idx_f[:])

            ones_col = cp.tile([P, 1], F32)
            nc.vector.memset(ones_col[:], 1.0)

            # ---------------- phase 1: embT [D, C] (fp32r) ----------------
            embT = cp.tile([P, KD, C], DT_SC)
            with tc.tile_pool(name="etmp", bufs=2) as tp:
                for kd in range(KD // 2):
                    t1 = tp.tile([P, C], F32, tag="t1")
                    nc.vector.tensor_scalar(
                        out=t1[:], in0=px, scalar1=fi_all[:, kd : kd + 1],
                        scalar2=None, op0=mybir.AluOpType.mult,
                    )
                    u = tp.tile([P, C], F32, tag="u")
                    nc.vector.tensor_scalar(
                        out=u[:], in0=py, scalar1=fj_all[:, kd : kd + 1],
                        scalar2=None, op0=mybir.AluOpType.mult,
                    )
                    nc.vector.tensor_tensor(
                        out=u[:], in0=u[:], in1=t1[:], op=mybir.AluOpType.add
                    )
                    # sin half: embT[kd + 8] = sin(2*pi*(u - rne(u)))
                    rn = tp.tile([P, C], F32, tag="rn")
                    nc.vector.tensor_scalar(
                        out=rn[:], in0=u[:], scalar1=MAGIC, scalar2=MAGIC,
                        op0=mybir.AluOpType.add, op1=mybir.AluOpType.subtract,
                    )
                    fr = tp.tile([P, C], F32, tag="fr")
                    nc.vector.tensor_tensor(
                        out=fr[:], in0=u[:], in1=rn[:], op=mybir.AluOpType.subtract
                    )
                    nc.scalar.activation(
                        embT[:, kd + KD // 2, :], fr[:],
                        mybir.ActivationFunctionType.Sin, bias=0.0, scale=TWO_PI,
                    )
                    # cos half: embT[kd] = sin(2*pi*(uc - rne(uc))), uc = u + 0.25
                    vc = tp.tile([P, C], F32, tag="vc")
                    nc.vector.tensor_scalar_add(out=vc[:], in0=u[:], scalar1=0.25)
                    rc = tp.tile([P, C], F32, tag="rc")
                    nc.vector.tensor_scalar(
                        out=rc[:], in0=vc[:], scalar1=MAGIC, scalar2=MAGIC,
                        op0=mybir.AluOpType.add, op1=mybir.AluOpType.subtract,
                    )
                    fc = tp.tile([P, C], F32, tag="fc")
                    nc.vector.tensor_tensor(
                        out=fc[:], in0=vc[:], in1=rc[:], op=mybir.AluOpType.subtract
                    )
                    nc.scalar.activation(
                        embT[:, kd, :], fc[:],
                        mybir.ActivationFunctionType.Sin, bias=0.0, scale=TWO_PI,
                    )

            # ---------------- phase 2: per-subject softmax weightsT ----------------
            with (
                tc.tile_pool(name="hT", bufs=S * KD) as h_p,
                tc.tile_pool(name="E", bufs=S) as e_p,
                tc.tile_pool(name="wT", bufs=2) as w_p,
                tc.tile_pool(name="smisc", bufs=4) as sm_p,
                tc.tile_pool(name="pssc", bufs=6, space="PSUM") as pssc_p,
            ):
                hts = {}
                for s in range(S):
                    for kd in range(KD):
                        hT = h_p.tile([P, O], F16, tag="hT", name=f"hT_{s}_{kd}")
                        nc.sync.dma_start(out=hT[:], in_=headsT[s, kd * P : (kd + 1) * P, :])
                        hts[(s, kd)] = hT
                e_ts = [e_p.tile([P, KC, O], F32, tag="E", name=f"E_{s}") for s in range(S)]
                prev_smm = None
                for mc in range(KC):
                    pk = _pk(mc)
                    pscs = [
                        pssc_p.tile([P, O], F32, space="PSUM", tag="psc", name=f"psc_{mc}_{s}")
                        for s in range(S)
                    ]
                    kd_order = [x for p in range(KD // 2) for x in (p, p + KD // 2)]
                    for ki, kd in enumerate(kd_order):
                        for s in range(S):
                            mm = nc.tensor.matmul(
                                out=pscs[s][:pk, :],
                                lhsT=embT[:, kd, mc * P : mc * P + pk],
                                rhs=hts[(s, kd)][:],
                                start=(ki == 0),
                                stop=(ki == KD - 1),
                            )
                            if prev_smm is not None:
                                add_dep_helper(mm.ins, prev_smm.ins, sync=False, reason="pe-order")
                            prev_smm = mm
                    for s in range(S):
                        nc.scalar.activation(
                            e_ts[s][:pk, mc, :], pscs[s][:pk, :],
                            mybir.ActivationFunctionType.Exp,
                        )
                with tc.tile_pool(name="pssum", bufs=1, space="PSUM") as pssum_p:
                    for s in range(S):
                        e_t = e_ts[s]
                        sums = pssum_p.tile([1, O], F32, space="PSUM", tag="sums", name=f"sums_{s}")
                        for mc in range(KC):
                            pk = _pk(mc)
                            nc.tensor.matmul(
                                out=sums[:, :],
                                lhsT=ones_col[:pk, :],
                                rhs=e_t[:pk, mc, :],
                                start=(mc == 0),
                                stop=(mc == KC - 1),
                            )
                        srow = sm_p.tile([1, O], F32, tag="srow", name=f"srow_{s}")
                        nc.vector.tensor_copy(out=srow[:], in_=sums[:])
                        rrow = sm_p.tile([1, O], F32, tag="rrow", name=f"rrow_{s}")
                        nc.vector.reciprocal(out=rrow[:], in_=srow[:])
                        recipB = sm_p.tile([P, O], F32, tag="recipB", name=f"recipB_{s}")
                        nc.gpsimd.partition_broadcast(recipB[:], rrow[:])
                        w_t = w_p.tile([P, KC, O], DT_MM, tag="wT", name=f"wT_{s}")
                        for mc in range(KC):
                            pk = _pk(mc)
                            nc.vector.tensor_tensor(
                                out=w_t[:pk, mc, :],
                                in0=e_t[:pk, mc, :],
                                in1=recipB[:pk, :],
                                op=mybir.AluOpType.mult,
                            )
                            nc.sync.dma_start(
                                out=scratch[s * KC * P + mc * P : s * KC * P + mc * P + pk, :],
                                in_=w_t[:pk, mc, :],
                            )

            # ---------------- phase 3: main einsum per batch slot ----------------
            psm_ctx = tc.tile_pool(name="psmain", bufs=8, space="PSUM")
            psm_p = psm_ctx.__enter__()
            prev_mm = None
            for j in range(BL):
                wsel = wsel_p.tile([P, KC, O], DT_MM, tag="wsel")
                for kc in range(KC):
                    pk = _pk(kc)
                    col = kc * BL + j
                    nc.gpsimd.indirect_dma_start(
                        out=wsel[:pk, kc, :],
                        out_offset=None,
                        in_=scratch[:],
                        in_offset=bass.IndirectOffsetOnAxis(
                            ap=idx_i[:pk, col : col + 1], axis=0
                        ),
                    )
                rhs = rhs_p.tile([P, KC, T], DT_MM, tag="rhs")
                # spread meg loads over both HWDGE rings (gpsimd keeps only
                # the indirect gathers, whose descriptor-gen is slow)
                for kc, eng in ((0, nc.sync), (1, nc.scalar), (2, nc.gpsimd)):
                    pk = _pk(kc)
                    eng.dma_start(
                        out=rhs[:pk, kc, :], in_=meg_l[j, kc * P : kc * P + pk, :]
                    )
                for mo in range(MO):
                    pmo = _pmo(mo)
                    pos = [
                        psm_p.tile([P, 512], F32, space="PSUM", tag="po", name=f"po_{j}_{mo}_{n}")
                        for n in range(NT)
                    ]
                    # kc outer / nt inner: one weight set serves NT consecutive matmuls
                    for kc in range(KC):
                        pk = _pk(kc)
                        for nt in range(NT):
                            mm = nc.tensor.matmul(
                                out=pos[nt][:pmo, :],
                                lhsT=wsel[:pk, kc, mo * P : mo * P + pmo],
                                rhs=rhs[:pk, kc, nt * 512 : (nt + 1) * 512],
                                start=(kc == 0),
                                stop=(kc == KC - 1),
                            )
                            # pin PE order kc-outer/nt-inner so the weight
                            # dedup pass sees adjacent identical LDWEIGHTS
                            if prev_mm is not None:
                                add_dep_helper(mm.ins, prev_mm.ins, sync=False, reason="pe-order")
                            prev_mm = mm
                    for nt in range(NT):
                        ot = ot_p.tile([P, 512], F32, tag="ot")
                        # alternate copy engines (DVE is mostly idle) and
                        # spread the out-DMAs over both HWDGE rings
                        if nt % 2 == 0:
                            nc.vector.tensor_copy(out=ot[:pmo, :], in_=pos[nt][:pmo, :])
                            nc.sync.dma_start(
                                out=out_l[j, mo * P : mo * P + pmo, nt * 512 : (nt + 1) * 512],
                                in_=ot[:pmo, :],
                            )
                        else:
                            nc.scalar.copy(out=ot[:pmo, :], in_=pos[nt][:pmo, :])
                            nc.scalar.dma_start(
                                out=out_l[j, mo * P : mo * P + pmo, nt * 512 : (nt + 1) * 512],
                                in_=ot[:pmo, :],
                            )

            psm_ctx.__exit__(None, None, None)

    import os as _os
    if not _os.environ.get("NO_DEDUP"):
        _dedupe_ldweights(nc)
    nc.compile()
    return nc


_NC = None


def _host_inputs():
    """Input arrays that do not depend on the problem inputs."""
    d = np.arange(KD * P // 2)
    ftab = np.stack(
        [
            ((d // 32).astype(np.float64) / WIDTH).astype(np.float32),
            ((d % 32).astype(np.float64) / WIDTH).astype(np.float32),
        ]
    )
    iota = np.arange(P, dtype=np.float32).reshape(P, 1)
    return ftab, iota


def kernel(meg, positions, heads, subject, _trace=False, **_unused):
    global _NC
    if _NC is None:
        _NC = build()
    nc = _NC

    meg = np.ascontiguousarray(meg, dtype=np.float16)
    positions = np.ascontiguousarray(positions, dtype=np.float32)
    headsT = np.ascontiguousarray(np.transpose(heads, (0, 2, 1)), dtype=np.float16)
    subject = np.asarray(subject).astype(np.int32)
    ftab, iota = _host_inputs()
    posflat = positions.reshape(1, 2 * C)

    in_maps = []
    for i in range(NCORES):
        in_maps.append(
            {
                "meg_l": meg[i * BL : (i + 1) * BL],
                "headsT": headsT,
                "posflat": posflat,
                "ftab": ftab,
                "iota": iota,
                "subj": subject[i * BL : (i + 1) * BL].reshape(1, BL),
            }
        )

    res = run_bass_kernel_spmd(nc, in_maps, list(range(NCORES)), trace=_trace)
    out = np.concatenate([res.results[i]["out_l"] for i in range(NCORES)], axis=0)
    if _trace:
        return out, res
    return out
